# revision 25
# baseline (speedup 1.0000x reference)
# BinaryLinear on 8 Trainium2 NeuronCores.
#
# y = x @ sign(W)^T + bias for x [8192, 4096] f32, W [4096, 4096] f32.
#
# Sharding: data-parallel over the 8192 tokens (1024 per core). Each core
# runs one [K=4096, M=1024] x [K=4096, N=4096] matmul entirely in fp8e4m3
# DoubleRow perf mode (the PE virtualizes to 128x256: 0.5 cycles/moving-row,
# 2x the bf16 FLOP rate, 4x per instruction), with a two-term x quantization:
#   - batch A: hi = e4m3(x), full K=4096, against full W.
#   - batch B: lo = e4m3(x - hi) on the first KLO = N_LO*256 columns,
#     against the same W rows (error-feedback correction).
# +-1 weights are exact in fp8; accumulation is fp32 in PSUM across both
# batches. Corrected columns carry only the second-order residual, so
# rel err ~= sqrt(1 - KLO/4096)*0.0266; N_LO=8 measures 0.0187 on the
# reference data vs the 2e-2 gate (all metric variants checked: Frobenius
# 0.0187, mean-abs 0.0188, scale-relative absmax 0.0144).
#
# All inputs are staged into SBUF up front (W 16.8MB + x-hi 4MB + x-lo
# 2.25MB fits alongside temps in the 24MB SBUF) with >=1MB consumption-
# ordered DMAs; the matmul tile producers are pure SBUF slices. The loop
# nest is swapped (swap_mm_args: W out-chunks outer, token tiles inner) so
# each staged W chunk is needed half as often while the DMA pipe drains,
# and PSUM is double-buffered (4 banks x 2) to remove block-boundary
# eviction stalls. Cost model: 189us/core vs 445us for the bf16 baseline.
#
# Execution goes through bass2jax/PJRT (axon): one jitted shard_map over the
# 8-core mesh. The donated output backing buffer is created on-device so no
# zero-filled bytes cross the host->device link.

import numpy as np
import ml_dtypes

N_TOKENS = 8192
IN_F = 4096
OUT_F = 4096
N_CORES = 8
TOK_SHARD = N_TOKENS // N_CORES

N_LO = 8  # number of 256-wide K chunks getting the lo correction term
KLO = N_LO * 256

_C = {}


OUT_DT = "float16"  # device-side output dtype (upcast to f32 on host).
# f16 keeps D2H small; rounding f32 PSUM results to f16 adds ~3e-4 relative
# error rms on top of the x-quantization error — negligible.


def _build_nc(
    out_dt=None,
    n_lo=None,
    # 256 gives K_SUBTILES=2, which is what lets the composable kernel emit
    # DoubleRow ([128,2,*] slices) for the fp8 batches.
    max_k_tile=256,
    max_tile=512,
    free_dim=512,
    repeats=1,
    psum_bufs=2,
    temps_bufs=6,
    n_warm=0,
    split_out=True,
    w_chunks=8,
    x_chunks=4,
):
    import concourse.mybir as mybir
    import concourse.tile as tile
    from concourse import bacc
    from concourse.kernels.tile_matmul import (
        ShapeInfo,
        batched_producer_kxm,
        batched_producer_kxn,
        composable_matmul_tile_kernel,
        dma_to_dram_mxn,
    )

    out_dt = out_dt or OUT_DT
    n_lo = N_LO if n_lo is None else n_lo
    klo = n_lo * 256
    KO, KOL = IN_F // 128, klo // 128
    nc = bacc.Bacc("TRN2", target_bir_lowering=False, debug=False)
    # All inputs arrive partition-major and consumption-chunk-major
    # (x by m-half, W by n-512-chunk) so each SBUF-staging DMA moves >=1MB
    # of per-partition-contiguous bytes, ordered to land just ahead of the
    # matmul stream's consumption.
    MCH, NCH = TOK_SHARD // 512, OUT_F // 512
    x8_t = nc.dram_tensor(
        "x8_t", [128, MCH * KO * 512], mybir.dt.float8e4, kind="ExternalInput"
    ).ap()
    xl_t = (
        nc.dram_tensor(
            "xl_t", [128, MCH * KOL * 512], mybir.dt.float8e4,
            kind="ExternalInput",
        ).ap()
        if klo
        else None
    )
    w_t = nc.dram_tensor(
        "w_t", [128, NCH * KO * 512], mybir.dt.float8e4, kind="ExternalInput"
    ).ap()
    y = nc.dram_tensor(
        "y", [TOK_SHARD, OUT_F], getattr(mybir.dt, out_dt), kind="ExternalOutput"
    ).ap()
    with tile.TileContext(nc) as tc:
        import contextlib

        with contextlib.ExitStack() as es:
            if n_warm:
                # PE warm-up: dependency-free dummy matmuls on memset tiles
                # run while the first input DMAs are in flight, so the real
                # matmul stream starts past the HAM/pstate ramp (the PE runs
                # at half clock until ~3.4us of sustained activity).
                warm = es.enter_context(tc.tile_pool(name="warm", bufs=1))
                warm_ps = es.enter_context(
                    tc.tile_pool(name="warm_ps", bufs=1, space="PSUM")
                )
                w_t_ = warm.tile([128, 512], mybir.dt.bfloat16)
                nc.vector.memset(w_t_[:], 1.0)
                w_out = warm_ps.tile([128, 512], mybir.dt.float32)
                for _ in range(n_warm):
                    nc.tensor.matmul(
                        w_out[:], w_t_[:, :128], w_t_[:], start=True, stop=True
                    )
            io_pool = es.enter_context(tc.tile_pool(name="io_pool", bufs=1))
            import concourse.bass as bass

            w4 = w_t.rearrange("p (nc ko n) -> p nc ko n", nc=NCH, ko=KO)
            x84 = x8_t.rearrange("p (mc ko m) -> p mc ko m", mc=MCH, ko=KO)
            xl4 = (
                xl_t.rearrange("p (mc ko m) -> p mc ko m", mc=MCH, ko=KOL)
                if klo
                else None
            )

            for _ in range(repeats):
                # SBUF-resident staging for all inputs (fp8: W 128KB/part,
                # x hi 32KB/part, x lo 18KB/part). DMAs are issued in the
                # matmul stream's consumption order (x/W k-chunks of the
                # first (m0,n0) block first, then W n-chunks, then the m1
                # half) so the PE is never waiting on staging after the
                # initial ~2MB fill.
                w_sbufs = [
                    io_pool.tile(
                        [128, KO, 512], mybir.dt.float8e4,
                        name=f"w_sbuf{c}", tag=f"w_sbuf{c}",
                    )
                    for c in range(NCH)
                ]
                x8_sbufs = [
                    io_pool.tile(
                        [128, KO, 512], mybir.dt.float8e4,
                        name=f"x8_sbuf{c}", tag=f"x8_sbuf{c}",
                    )
                    for c in range(MCH)
                ]
                xl_sbufs = [
                    (
                        io_pool.tile(
                            [128, KOL, 512], mybir.dt.float8e4,
                            name=f"xl_sbuf{c}", tag=f"xl_sbuf{c}",
                        )
                        if klo
                        else None
                    )
                    for c in range(MCH)
                ]

                def dma(sb, dr, mi, k0, k1):
                    nc.sync.dma_start(sb[:, k0:k1, :], dr[:, mi, k0:k1, :])

                KH = KO // 2
                dma(x8_sbufs[0], x84, 0, 0, KH)      # x hi, m0, k first half
                dma(w_sbufs[0], w4, 0, 0, KH)        # W n0, k first half
                dma(x8_sbufs[0], x84, 0, KH, KO)     # x hi, m0, k second half
                dma(w_sbufs[0], w4, 0, KH, KO)       # W n0, k second half
                if klo:
                    dma(xl_sbufs[0], xl4, 0, 0, KOL)  # x lo, m0
                dma(x8_sbufs[1], x84, 1, 0, KH)      # x hi, m1
                dma(x8_sbufs[1], x84, 1, KH, KO)
                dma(w_sbufs[1], w4, 1, 0, KO)        # W n1
                if klo:
                    dma(xl_sbufs[1], xl4, 1, 0, KOL)  # x lo, m1
                for n in range(2, NCH):
                    dma(w_sbufs[n], w4, n, 0, KO)    # W n2..n7

                # swap_mm_args: kxm (outer loop) = W, kxn (inner) = x.
                # Same matmuls/psum layout as unswapped, but each W n-chunk
                # is needed half as often, doubling the staging slack per
                # chunk against the saturated DMA pipe.
                def make_w_prod():
                    def producer(nc_, md):
                        ks = md.k_tile_idx * md.k_subtiles
                        return w_sbufs[md.m_tile_idx][
                            :, ks : ks + md.k_subtiles, :
                        ]

                    return producer

                def make_x_prod(sbufs):
                    def producer(nc_, md):
                        ks = md.k_tile_idx * md.k_subtiles
                        return sbufs[md.n_tile_idx][
                            :, ks : ks + md.k_subtiles, :
                        ]

                    return producer

                kxm_prods = [make_w_prod()]
                kxm_shapes = [
                    ShapeInfo(pdims=((128, KO),), fdims=(OUT_F - 512,))
                ]
                kxn_prods = [make_x_prod(x8_sbufs)]
                kxn_shapes = [ShapeInfo(pdims=((128, KO),), fdims=(TOK_SHARD,))]
                if klo:
                    kxm_prods.append(make_w_prod())
                    kxm_shapes.append(
                        ShapeInfo(pdims=((128, KOL),), fdims=(OUT_F - 512,))
                    )
                    kxn_prods.append(make_x_prod(xl_sbufs))
                    kxn_shapes.append(
                        ShapeInfo(pdims=((128, KOL),), fdims=(TOK_SHARD,))
                    )
                kxm_producer, kxm_shape = batched_producer_kxm(
                    kxm_prods, kxm_shapes, batch_dim="k"
                )
                kxn_producer, kxn_shape = batched_producer_kxn(
                    kxn_prods, kxn_shapes, batch_dim="k"
                )
                y3 = y.rearrange("(po pi) f -> pi po f", pi=128)
                extra = {}
                if split_out:
                    # Evict+store per PSUM subtile: each subtile's DRAM DMA
                    # starts right after its own PSUM->SBUF copy, spreading
                    # store DMAs across the block instead of bunching them
                    # (measured faster on-device than one batched store;
                    # cost-model delta is negligible).

                    def reducer(nc_, psum, sbuf, md):
                        # alternate engines so the block's 4 evictions run
                        # pairwise-parallel (GpSimd can't read PSUM);
                        # swapped-args metadata: m_* indexes W out-chunks,
                        # n_* indexes token tiles; psum is [128 tok, 512 out]
                        if md.m_subtile_idx % 2 == 0:
                            nc_.vector.tensor_copy(out=sbuf, in_=psum)
                        else:
                            nc_.scalar.copy(out=sbuf, in_=psum)
                        po = md.n_tile_idx * md.m_subtiles + md.m_subtile_idx
                        nc_.sync.dma_start(
                            y3[:, po, bass.ds(md.m_tile_idx * md.m_tile, 512)],
                            sbuf[:, 0, :512],
                        )

                    extra["mxn_subtile_reducer"] = reducer
                    mxn_consumer = lambda nc_, tile_, md: None
                else:
                    mxn_consumer = dma_to_dram_mxn(y)
                composable_matmul_tile_kernel(
                    tc=tc,
                    kxm_shape=kxm_shape,
                    kxn_shape=kxn_shape,
                    output_type=y.dtype,
                    kxm_producer=kxm_producer,
                    kxn_producer=kxn_producer,
                    mxn_consumer=mxn_consumer,
                    MATMUL_FREE_DIM=free_dim,
                    MAX_TILE_SIZE=max_tile,
                    MAX_K_TILE_SIZE=max_k_tile,
                    temps_n_bufs=temps_bufs,
                    psum_n_bufs=psum_bufs,
                    swap_mm_args=True,
                    skip_k_snake=True,
                    **extra,
                )
                # Peeled epilogue: the last W chunk runs k-innermost per
                # token-subtile, so each subtile's eviction+store overlaps
                # the next subtile's matmuls instead of all four draining
                # after the final matmul (shrinks the kernel tail).
                with contextlib.ExitStack() as ees:
                    eps = ees.enter_context(
                        tc.tile_pool(name="epi_ps", bufs=1, space="PSUM")
                    )
                    esb = ees.enter_context(
                        tc.tile_pool(name="epi_sb", bufs=2)
                    )
                    wlast = w_sbufs[NCH - 1]
                    for half in range(MCH):
                        for ms in range(4):
                            pt = eps.tile(
                                [128, 512], mybir.dt.float32,
                                name=f"ep_{half}_{ms}", tag=f"ep_{ms}",
                            )
                            for r in range(KO // 2):
                                nc.tensor.matmul(
                                    pt[:],
                                    x8_sbufs[half][
                                        :, 2 * r : 2 * r + 2,
                                        ms * 128 : (ms + 1) * 128,
                                    ],
                                    wlast[:, 2 * r : 2 * r + 2, :],
                                    start=(r == 0),
                                    stop=False,
                                    perf_mode=mybir.MatmulPerfMode.DoubleRow,
                                )
                            for r in range(KOL // 2):
                                nc.tensor.matmul(
                                    pt[:],
                                    xl_sbufs[half][
                                        :, 2 * r : 2 * r + 2,
                                        ms * 128 : (ms + 1) * 128,
                                    ],
                                    wlast[:, 2 * r : 2 * r + 2, :],
                                    start=False,
                                    stop=(r == KOL // 2 - 1),
                                    perf_mode=mybir.MatmulPerfMode.DoubleRow,
                                )
                            sb = esb.tile(
                                [128, 512], getattr(mybir.dt, out_dt),
                                name=f"eo_{half}_{ms}", tag="eo",
                            )
                            if ms % 2 == 0:
                                nc.vector.tensor_copy(out=sb[:], in_=pt[:])
                            else:
                                nc.scalar.copy(out=sb[:], in_=pt[:])
                            nc.sync.dma_start(
                                y3[
                                    :,
                                    half * 4 + ms,
                                    bass.ds(OUT_F - 512, 512),
                                ],
                                sb[:],
                            )
    nc.compile()
    return nc


def _get_nc():
    if "nc" not in _C:
        _C["nc"] = _build_nc()
    return _C["nc"]


def _in_names(nc):
    import concourse.mybir as mybir

    partition_name = nc.partition_id_tensor.name if nc.partition_id_tensor else None
    names = []
    for alloc in nc.m.functions[0].allocations:
        if not isinstance(alloc, mybir.MemoryLocationSet):
            continue
        name = alloc.memorylocations[0].name
        if alloc.kind == "ExternalInput" and name != partition_name:
            names.append(name)
    return names


def _get_runner():
    """Compile the 8-core jitted executable once; returns (fn, zeros_fn)."""
    if "runner" in _C:
        return _C["runner"]
    import jax
    import jax.numpy as jnp
    from jax.sharding import Mesh, NamedSharding, PartitionSpec

    import inspect

    try:
        from jax.experimental.shard_map import shard_map
    except ImportError:
        from jax import shard_map
    _rep_kw = (
        {"check_rep": False}
        if "check_rep" in inspect.signature(shard_map).parameters
        else {"check_vma": False}
    )
    import concourse.mybir as mybir
    from concourse import bass2jax
    from concourse.bass2jax import _bass_exec_p, install_neuronx_cc_hook

    nc = _get_nc()
    install_neuronx_cc_hook()

    partition_name = nc.partition_id_tensor.name if nc.partition_id_tensor else None
    in_names, out_names, out_avals = [], [], []
    for alloc in nc.m.functions[0].allocations:
        if not isinstance(alloc, mybir.MemoryLocationSet):
            continue
        name = alloc.memorylocations[0].name
        if alloc.kind == "ExternalInput":
            if name != partition_name:
                in_names.append(name)
        elif alloc.kind == "ExternalOutput":
            out_names.append(name)
            out_avals.append(
                jax.core.ShapedArray(
                    tuple(alloc.tensor_shape), mybir.dt.np(alloc.dtype)
                )
            )
    expect = ["x8_t"] + (["xl_t"] if KLO else []) + ["w_t"]
    assert in_names == expect and out_names == ["y"], (in_names, out_names)
    all_in_names = list(in_names) + list(out_names)
    if partition_name is not None:
        all_in_names.append(partition_name)

    def _body(*args):
        operands = list(args)
        if partition_name is not None:
            operands.append(bass2jax.partition_id_tensor())
        outs = _bass_exec_p.bind(
            *operands,
            out_avals=tuple(out_avals),
            in_names=tuple(all_in_names),
            out_names=tuple(out_names),
            lowering_input_output_aliases=(),
            sim_require_finite=True,
            sim_require_nnan=True,
            nc=nc,
        )
        return tuple(outs)

    devices = jax.devices()[:N_CORES]
    mesh = Mesh(np.asarray(devices), ("core",))
    sharding = NamedSharding(mesh, PartitionSpec("core"))
    n_args = len(in_names) + 1  # inputs + y backing
    in_specs = (PartitionSpec("core"),) * n_args
    out_specs = (PartitionSpec("core"),)
    fn = jax.jit(
        shard_map(_body, mesh=mesh, in_specs=in_specs, out_specs=out_specs,
                  **_rep_kw),
        donate_argnums=(n_args - 1,),
        keep_unused=True,
    )
    out_np_dt = out_avals[0].dtype
    zeros_fn = jax.jit(
        lambda: jnp.zeros((N_TOKENS, OUT_F), out_np_dt),
        out_shardings=sharding,
    )
    _C["runner"] = (fn, zeros_fn, sharding, jax)
    return _C["runner"]


def _pm(a):
    """[K, M] -> [128, (M//512)*(K//128)*512]: partition-major with the
    free dim chunked by 512 outermost (the kernel's staging-DMA order)."""
    K, M = a.shape
    t = a.reshape(K // 128, 128, M // 512, 512).transpose(1, 2, 0, 3)
    return np.ascontiguousarray(t.reshape(128, -1))


def _shard_cols_pm(xt):
    """[K, 8192] -> [8*128, ...] stacked per-core chunk-major shards."""
    return np.concatenate(
        [_pm(xt[:, c * TOK_SHARD : (c + 1) * TOK_SHARD]) for c in range(N_CORES)],
        axis=0,
    )


def _host_prep(x, weight):
    """sign/transpose/cast/shard on the host (cheap vs the matmul).

    Returns the global (8-core stacked) arrays in kernel input order:
    x8_t (fp8 K-slice), xb_t (bf16 K-slice), w_t.
    """
    xt = np.ascontiguousarray(np.asarray(x).T)
    hi = xt.astype(ml_dtypes.float8_e4m3)
    parts = [_shard_cols_pm(hi)]
    if KLO:
        lo = (xt[:KLO] - hi[:KLO].astype(np.float32)).astype(
            ml_dtypes.float8_e4m3
        )
        parts.append(_shard_cols_pm(lo))
    wt = _pm(
        np.sign(np.asarray(weight)).T.astype(ml_dtypes.float8_e4m3)
    )
    parts.append(np.concatenate([wt] * N_CORES, axis=0))
    return parts


def _run_spmd_fallback(x, weight):
    """Conservative path through bass_utils.run_bass_kernel_spmd (same
    underlying bass2jax/PJRT execution; pays extra host->device bytes for the
    zero-filled output backing buffers)."""
    from concourse.bass_utils import run_bass_kernel_spmd

    nc = _get_nc()
    xt = np.ascontiguousarray(np.asarray(x).T)
    hi = xt.astype(ml_dtypes.float8_e4m3)
    lo = (
        (xt[:KLO] - hi[:KLO].astype(np.float32)).astype(ml_dtypes.float8_e4m3)
        if KLO
        else None
    )
    wt = _pm(np.sign(np.asarray(weight)).T.astype(ml_dtypes.float8_e4m3))
    in_maps = []
    for c in range(N_CORES):
        sl = slice(c * TOK_SHARD, (c + 1) * TOK_SHARD)
        m = {"x8_t": _pm(hi[:, sl])}
        if lo is not None:
            m["xl_t"] = _pm(lo[:, sl])
        m["w_t"] = wt
        in_maps.append(m)
    res = run_bass_kernel_spmd(nc, in_maps, core_ids=list(range(N_CORES)))
    return np.concatenate([r["y"] for r in res.results], axis=0)


def kernel(x, weight, bias):
    try:
        fn, zeros_fn, sharding, jax = _get_runner()
        parts = _host_prep(x, weight)
        args = [jax.device_put(p, sharding) for p in parts]
        args.append(zeros_fn())
        (yd,) = fn(*args)
        # global [8192, 4096], token order preserved
        y = np.asarray(yd)
    except Exception:
        y = _run_spmd_fallback(x, weight)
    # upcast + bias on host
    y = y.astype(np.float32)
    y += np.asarray(bias, dtype=np.float32)[None, :]
    return y


# revision 27
# speedup vs baseline: 1.1865x; 1.1865x over previous
# BinaryLinear on 8 Trainium2 NeuronCores.
#
# y = x @ sign(W)^T + bias for x [8192, 4096] f32, W [4096, 4096] f32.
#
# Sharding: data-parallel over the 8192 tokens (1024 per core). Each core
# runs one [K=4096, M=1024] x [K=4096, N=4096] matmul entirely in fp8e4m3
# DoubleRow perf mode (the PE virtualizes to 128x256: 0.5 cycles/moving-row,
# 2x the bf16 FLOP rate, 4x per instruction), with a two-term x quantization:
#   - batch A: hi = e4m3(x), full K=4096, against full W.
#   - batch B: lo = e4m3(x - hi) on the first KLO = N_LO*256 columns,
#     against the same W rows (error-feedback correction).
# +-1 weights are exact in fp8; accumulation is fp32 in PSUM across both
# batches. Corrected columns carry only the second-order residual, so
# rel err ~= sqrt(1 - KLO/4096)*0.0266; N_LO=8 measures 0.0187 on the
# reference data vs the 2e-2 gate (all metric variants checked: Frobenius
# 0.0187, mean-abs 0.0188, scale-relative absmax 0.0144).
#
# All inputs are staged into SBUF up front (W 16.8MB + x-hi 4MB + x-lo
# 2.25MB fits alongside temps in the 24MB SBUF) with >=1MB consumption-
# ordered DMAs; the matmul tile producers are pure SBUF slices. The loop
# nest is swapped (swap_mm_args: W out-chunks outer, token tiles inner) so
# each staged W chunk is needed half as often while the DMA pipe drains,
# and PSUM is double-buffered (4 banks x 2) to remove block-boundary
# eviction stalls. Cost model: 189us/core vs 445us for the bf16 baseline.
#
# Execution goes through bass2jax/PJRT (axon): one jitted shard_map over the
# 8-core mesh. The donated output backing buffer is created on-device so no
# zero-filled bytes cross the host->device link.

import numpy as np
import ml_dtypes

N_TOKENS = 8192
IN_F = 4096
OUT_F = 4096
N_CORES = 8
TOK_SHARD = N_TOKENS // N_CORES

N_LO = 8  # number of 256-wide K chunks getting the lo correction term
KLO = N_LO * 256

_C = {}


OUT_DT = "float16"  # device-side output dtype (upcast to f32 on host).
# f16 keeps D2H small; rounding f32 PSUM results to f16 adds ~3e-4 relative
# error rms on top of the x-quantization error — negligible.


def _build_nc(
    out_dt=None,
    n_lo=None,
    # 256 gives K_SUBTILES=2, which is what lets the composable kernel emit
    # DoubleRow ([128,2,*] slices) for the fp8 batches.
    max_k_tile=256,
    max_tile=512,
    free_dim=512,
    repeats=1,
    psum_bufs=2,
    temps_bufs=6,
    n_warm=0,
    split_out=True,
    w_chunks=8,
    x_chunks=4,
):
    import concourse.mybir as mybir
    import concourse.tile as tile
    from concourse import bacc
    from concourse.kernels.tile_matmul import (
        ShapeInfo,
        batched_producer_kxm,
        batched_producer_kxn,
        composable_matmul_tile_kernel,
        dma_to_dram_mxn,
    )

    out_dt = out_dt or OUT_DT
    n_lo = N_LO if n_lo is None else n_lo
    klo = n_lo * 256
    KO, KOL = IN_F // 128, klo // 128
    nc = bacc.Bacc("TRN2", target_bir_lowering=False, debug=False)
    # All inputs arrive partition-major and consumption-chunk-major
    # (x by m-half, W by n-512-chunk) so each SBUF-staging DMA moves >=1MB
    # of per-partition-contiguous bytes, ordered to land just ahead of the
    # matmul stream's consumption.
    MCH, NCH = TOK_SHARD // 512, OUT_F // 512
    x8_t = nc.dram_tensor(
        "x8_t", [128, MCH * KO * 512], mybir.dt.float8e4, kind="ExternalInput"
    ).ap()
    xl_t = (
        nc.dram_tensor(
            "xl_t", [128, MCH * KOL * 512], mybir.dt.float8e4,
            kind="ExternalInput",
        ).ap()
        if klo
        else None
    )
    w_t = nc.dram_tensor(
        "w_t", [128, NCH * KO * 512], mybir.dt.float8e4, kind="ExternalInput"
    ).ap()
    y = nc.dram_tensor(
        "y", [TOK_SHARD, OUT_F], getattr(mybir.dt, out_dt), kind="ExternalOutput"
    ).ap()
    with tile.TileContext(nc) as tc:
        import contextlib

        with contextlib.ExitStack() as es:
            if n_warm:
                # PE warm-up: dependency-free dummy matmuls on memset tiles
                # run while the first input DMAs are in flight, so the real
                # matmul stream starts past the HAM/pstate ramp (the PE runs
                # at half clock until ~3.4us of sustained activity).
                warm = es.enter_context(tc.tile_pool(name="warm", bufs=1))
                warm_ps = es.enter_context(
                    tc.tile_pool(name="warm_ps", bufs=1, space="PSUM")
                )
                w_t_ = warm.tile([128, 512], mybir.dt.bfloat16)
                nc.vector.memset(w_t_[:], 1.0)
                w_out = warm_ps.tile([128, 512], mybir.dt.float32)
                for _ in range(n_warm):
                    nc.tensor.matmul(
                        w_out[:], w_t_[:, :128], w_t_[:], start=True, stop=True
                    )
            io_pool = es.enter_context(tc.tile_pool(name="io_pool", bufs=1))
            import concourse.bass as bass

            w4 = w_t.rearrange("p (nc ko n) -> p nc ko n", nc=NCH, ko=KO)
            x84 = x8_t.rearrange("p (mc ko m) -> p mc ko m", mc=MCH, ko=KO)
            xl4 = (
                xl_t.rearrange("p (mc ko m) -> p mc ko m", mc=MCH, ko=KOL)
                if klo
                else None
            )

            for _ in range(repeats):
                # SBUF-resident staging for all inputs (fp8: W 128KB/part,
                # x hi 32KB/part, x lo 18KB/part). DMAs are issued in the
                # matmul stream's consumption order (x/W k-chunks of the
                # first (m0,n0) block first, then W n-chunks, then the m1
                # half) so the PE is never waiting on staging after the
                # initial ~2MB fill.
                w_sbufs = [
                    io_pool.tile(
                        [128, KO, 512], mybir.dt.float8e4,
                        name=f"w_sbuf{c}", tag=f"w_sbuf{c}",
                    )
                    for c in range(NCH)
                ]
                x8_sbufs = [
                    io_pool.tile(
                        [128, KO, 512], mybir.dt.float8e4,
                        name=f"x8_sbuf{c}", tag=f"x8_sbuf{c}",
                    )
                    for c in range(MCH)
                ]
                xl_sbufs = [
                    (
                        io_pool.tile(
                            [128, KOL, 512], mybir.dt.float8e4,
                            name=f"xl_sbuf{c}", tag=f"xl_sbuf{c}",
                        )
                        if klo
                        else None
                    )
                    for c in range(MCH)
                ]

                def dma(sb, dr, mi, k0, k1):
                    nc.sync.dma_start(sb[:, k0:k1, :], dr[:, mi, k0:k1, :])

                KH = KO // 2
                dma(x8_sbufs[0], x84, 0, 0, KH)      # x hi, m0, k first half
                dma(w_sbufs[0], w4, 0, 0, KH)        # W n0, k first half
                dma(x8_sbufs[0], x84, 0, KH, KO)     # x hi, m0, k second half
                dma(w_sbufs[0], w4, 0, KH, KO)       # W n0, k second half
                if klo:
                    dma(xl_sbufs[0], xl4, 0, 0, KOL)  # x lo, m0
                dma(x8_sbufs[1], x84, 1, 0, KH)      # x hi, m1
                dma(x8_sbufs[1], x84, 1, KH, KO)
                dma(w_sbufs[1], w4, 1, 0, KO)        # W n1
                if klo:
                    dma(xl_sbufs[1], xl4, 1, 0, KOL)  # x lo, m1
                for n in range(2, NCH):
                    dma(w_sbufs[n], w4, n, 0, KO)    # W n2..n7

                # swap_mm_args: kxm (outer loop) = W, kxn (inner) = x.
                # Same matmuls/psum layout as unswapped, but each W n-chunk
                # is needed half as often, doubling the staging slack per
                # chunk against the saturated DMA pipe.
                def make_w_prod():
                    def producer(nc_, md):
                        ks = md.k_tile_idx * md.k_subtiles
                        return w_sbufs[md.m_tile_idx][
                            :, ks : ks + md.k_subtiles, :
                        ]

                    return producer

                def make_x_prod(sbufs):
                    def producer(nc_, md):
                        ks = md.k_tile_idx * md.k_subtiles
                        return sbufs[md.n_tile_idx][
                            :, ks : ks + md.k_subtiles, :
                        ]

                    return producer

                kxm_prods = [make_w_prod()]
                kxm_shapes = [
                    ShapeInfo(pdims=((128, KO),), fdims=(OUT_F - 512,))
                ]
                kxn_prods = [make_x_prod(x8_sbufs)]
                kxn_shapes = [ShapeInfo(pdims=((128, KO),), fdims=(TOK_SHARD,))]
                if klo:
                    kxm_prods.append(make_w_prod())
                    kxm_shapes.append(
                        ShapeInfo(pdims=((128, KOL),), fdims=(OUT_F - 512,))
                    )
                    kxn_prods.append(make_x_prod(xl_sbufs))
                    kxn_shapes.append(
                        ShapeInfo(pdims=((128, KOL),), fdims=(TOK_SHARD,))
                    )
                kxm_producer, kxm_shape = batched_producer_kxm(
                    kxm_prods, kxm_shapes, batch_dim="k"
                )
                kxn_producer, kxn_shape = batched_producer_kxn(
                    kxn_prods, kxn_shapes, batch_dim="k"
                )
                y3 = y.rearrange("(po pi) f -> pi po f", pi=128)
                extra = {}
                if split_out:
                    # Evict+store per PSUM subtile: each subtile's DRAM DMA
                    # starts right after its own PSUM->SBUF copy, spreading
                    # store DMAs across the block instead of bunching them
                    # (measured faster on-device than one batched store;
                    # cost-model delta is negligible).

                    def reducer(nc_, psum, sbuf, md):
                        # alternate engines so the block's 4 evictions run
                        # pairwise-parallel (GpSimd can't read PSUM);
                        # swapped-args metadata: m_* indexes W out-chunks,
                        # n_* indexes token tiles; psum is [128 tok, 512 out]
                        if md.m_subtile_idx % 2 == 0:
                            nc_.vector.tensor_copy(out=sbuf, in_=psum)
                        else:
                            nc_.scalar.copy(out=sbuf, in_=psum)
                        po = md.n_tile_idx * md.m_subtiles + md.m_subtile_idx
                        nc_.sync.dma_start(
                            y3[:, po, bass.ds(md.m_tile_idx * md.m_tile, 512)],
                            sbuf[:, 0, :512],
                        )

                    extra["mxn_subtile_reducer"] = reducer
                    mxn_consumer = lambda nc_, tile_, md: None
                else:
                    mxn_consumer = dma_to_dram_mxn(y)
                composable_matmul_tile_kernel(
                    tc=tc,
                    kxm_shape=kxm_shape,
                    kxn_shape=kxn_shape,
                    output_type=y.dtype,
                    kxm_producer=kxm_producer,
                    kxn_producer=kxn_producer,
                    mxn_consumer=mxn_consumer,
                    MATMUL_FREE_DIM=free_dim,
                    MAX_TILE_SIZE=max_tile,
                    MAX_K_TILE_SIZE=max_k_tile,
                    temps_n_bufs=temps_bufs,
                    psum_n_bufs=psum_bufs,
                    swap_mm_args=True,
                    skip_k_snake=True,
                    **extra,
                )
                # Peeled epilogue: the last W chunk runs k-innermost per
                # token-subtile, so each subtile's eviction+store overlaps
                # the next subtile's matmuls instead of all four draining
                # after the final matmul (shrinks the kernel tail).
                with contextlib.ExitStack() as ees:
                    eps = ees.enter_context(
                        tc.tile_pool(name="epi_ps", bufs=1, space="PSUM")
                    )
                    esb = ees.enter_context(
                        tc.tile_pool(name="epi_sb", bufs=2)
                    )
                    wlast = w_sbufs[NCH - 1]
                    for half in range(MCH):
                        for ms in range(4):
                            pt = eps.tile(
                                [128, 512], mybir.dt.float32,
                                name=f"ep_{half}_{ms}", tag=f"ep_{ms}",
                            )
                            for r in range(KO // 2):
                                nc.tensor.matmul(
                                    pt[:],
                                    x8_sbufs[half][
                                        :, 2 * r : 2 * r + 2,
                                        ms * 128 : (ms + 1) * 128,
                                    ],
                                    wlast[:, 2 * r : 2 * r + 2, :],
                                    start=(r == 0),
                                    stop=False,
                                    perf_mode=mybir.MatmulPerfMode.DoubleRow,
                                )
                            for r in range(KOL // 2):
                                nc.tensor.matmul(
                                    pt[:],
                                    xl_sbufs[half][
                                        :, 2 * r : 2 * r + 2,
                                        ms * 128 : (ms + 1) * 128,
                                    ],
                                    wlast[:, 2 * r : 2 * r + 2, :],
                                    start=False,
                                    stop=(r == KOL // 2 - 1),
                                    perf_mode=mybir.MatmulPerfMode.DoubleRow,
                                )
                            sb = esb.tile(
                                [128, 512], getattr(mybir.dt, out_dt),
                                name=f"eo_{half}_{ms}", tag="eo",
                            )
                            if ms % 2 == 0:
                                nc.vector.tensor_copy(out=sb[:], in_=pt[:])
                            else:
                                nc.scalar.copy(out=sb[:], in_=pt[:])
                            nc.sync.dma_start(
                                y3[
                                    :,
                                    half * 4 + ms,
                                    bass.ds(OUT_F - 512, 512),
                                ],
                                sb[:],
                            )
    nc.compile()
    return nc


def _get_nc():
    if "nc" not in _C:
        _C["nc"] = _build_nc()
    return _C["nc"]


def _in_names(nc):
    import concourse.mybir as mybir

    partition_name = nc.partition_id_tensor.name if nc.partition_id_tensor else None
    names = []
    for alloc in nc.m.functions[0].allocations:
        if not isinstance(alloc, mybir.MemoryLocationSet):
            continue
        name = alloc.memorylocations[0].name
        if alloc.kind == "ExternalInput" and name != partition_name:
            names.append(name)
    return names


def _get_runner():
    """Compile the 8-core jitted executable once; returns (fn, zeros_fn)."""
    if "runner" in _C:
        return _C["runner"]
    import jax
    import jax.numpy as jnp
    from jax.sharding import Mesh, NamedSharding, PartitionSpec

    import inspect

    try:
        from jax.experimental.shard_map import shard_map
    except ImportError:
        from jax import shard_map
    _rep_kw = (
        {"check_rep": False}
        if "check_rep" in inspect.signature(shard_map).parameters
        else {"check_vma": False}
    )
    import concourse.mybir as mybir
    from concourse import bass2jax
    from concourse.bass2jax import _bass_exec_p, install_neuronx_cc_hook

    nc = _get_nc()
    install_neuronx_cc_hook()

    partition_name = nc.partition_id_tensor.name if nc.partition_id_tensor else None
    in_names, out_names, out_avals = [], [], []
    for alloc in nc.m.functions[0].allocations:
        if not isinstance(alloc, mybir.MemoryLocationSet):
            continue
        name = alloc.memorylocations[0].name
        if alloc.kind == "ExternalInput":
            if name != partition_name:
                in_names.append(name)
        elif alloc.kind == "ExternalOutput":
            out_names.append(name)
            out_avals.append(
                jax.core.ShapedArray(
                    tuple(alloc.tensor_shape), mybir.dt.np(alloc.dtype)
                )
            )
    expect = ["x8_t"] + (["xl_t"] if KLO else []) + ["w_t"]
    assert in_names == expect and out_names == ["y"], (in_names, out_names)
    all_in_names = list(in_names) + list(out_names)
    if partition_name is not None:
        all_in_names.append(partition_name)

    def _body(*args):
        operands = list(args)
        if partition_name is not None:
            operands.append(bass2jax.partition_id_tensor())
        outs = _bass_exec_p.bind(
            *operands,
            out_avals=tuple(out_avals),
            in_names=tuple(all_in_names),
            out_names=tuple(out_names),
            lowering_input_output_aliases=(),
            sim_require_finite=True,
            sim_require_nnan=True,
            nc=nc,
        )
        return tuple(outs)

    devices = jax.devices()[:N_CORES]
    mesh = Mesh(np.asarray(devices), ("core",))
    sharding = NamedSharding(mesh, PartitionSpec("core"))
    n_args = len(in_names) + 1  # inputs + y backing
    in_specs = (PartitionSpec("core"),) * n_args
    out_specs = (PartitionSpec("core"),)
    fn = jax.jit(
        shard_map(_body, mesh=mesh, in_specs=in_specs, out_specs=out_specs,
                  **_rep_kw),
        donate_argnums=(n_args - 1,),
        keep_unused=True,
    )
    out_np_dt = out_avals[0].dtype
    zeros_fn = jax.jit(
        lambda: jnp.zeros((N_TOKENS, OUT_F), out_np_dt),
        out_shardings=sharding,
    )
    _C["runner"] = (fn, zeros_fn, sharding, jax)
    return _C["runner"]


def _pm(a):
    """[K, M] -> [128, (M//512)*(K//128)*512]: partition-major with the
    free dim chunked by 512 outermost (the kernel's staging-DMA order)."""
    K, M = a.shape
    t = a.reshape(K // 128, 128, M // 512, 512).transpose(1, 2, 0, 3)
    return np.ascontiguousarray(t.reshape(128, -1))


def _shard_cols_pm(xt):
    """[K, 8192] -> [8*128, ...] stacked per-core chunk-major shards."""
    return np.concatenate(
        [_pm(xt[:, c * TOK_SHARD : (c + 1) * TOK_SHARD]) for c in range(N_CORES)],
        axis=0,
    )


def _host_prep(x, weight):
    """sign/transpose/cast/shard on the host (cheap vs the matmul).

    Returns the global (8-core stacked) arrays in kernel input order:
    x8_t (fp8 K-slice), xb_t (bf16 K-slice), w_t.
    """
    xt = np.ascontiguousarray(np.asarray(x).T)
    hi = xt.astype(ml_dtypes.float8_e4m3)
    parts = [_shard_cols_pm(hi)]
    if KLO:
        lo = (xt[:KLO] - hi[:KLO].astype(np.float32)).astype(
            ml_dtypes.float8_e4m3
        )
        parts.append(_shard_cols_pm(lo))
    wt = _pm(
        np.sign(np.asarray(weight)).T.astype(ml_dtypes.float8_e4m3)
    )
    parts.append(np.concatenate([wt] * N_CORES, axis=0))
    return parts


def _run_spmd_fallback(x, weight):
    """Conservative path through bass_utils.run_bass_kernel_spmd (same
    underlying bass2jax/PJRT execution; pays extra host->device bytes for the
    zero-filled output backing buffers)."""
    from concourse.bass_utils import run_bass_kernel_spmd

    nc = _get_nc()
    xt = np.ascontiguousarray(np.asarray(x).T)
    hi = xt.astype(ml_dtypes.float8_e4m3)
    lo = (
        (xt[:KLO] - hi[:KLO].astype(np.float32)).astype(ml_dtypes.float8_e4m3)
        if KLO
        else None
    )
    wt = _pm(np.sign(np.asarray(weight)).T.astype(ml_dtypes.float8_e4m3))
    in_maps = []
    for c in range(N_CORES):
        sl = slice(c * TOK_SHARD, (c + 1) * TOK_SHARD)
        m = {"x8_t": _pm(hi[:, sl])}
        if lo is not None:
            m["xl_t"] = _pm(lo[:, sl])
        m["w_t"] = wt
        in_maps.append(m)
    res = run_bass_kernel_spmd(nc, in_maps, core_ids=list(range(N_CORES)))
    return np.concatenate([r["y"] for r in res.results], axis=0)


def kernel(x, weight, bias):
    try:
        fn, zeros_fn, sharding, jax = _get_runner()
        parts = _host_prep(x, weight)
        args = [jax.device_put(p, sharding) for p in parts]
        args.append(zeros_fn())
        (yd,) = fn(*args)
        # global [8192, 4096], token order preserved
        y = np.asarray(yd)
    except Exception:
        y = _run_spmd_fallback(x, weight)
    # upcast + bias on host
    y = y.astype(np.float32)
    y += np.asarray(bias, dtype=np.float32)[None, :]
    return y
